# revision 73
# baseline (speedup 1.0000x reference)
"""Trainium2 Bass kernel: feature-attention (dense_transformer).

    score = softmax((q^T @ k) / sqrt(H), axis=-1)   # (B,H,D,D), contraction over S
    out   = score @ v^T                              # (B,H,D,S)

q,k,v: (4,16,4096,128) f32.  B*H = 64 head-pairs sharded 8-per-core across
8 NeuronCores (pure data/head parallelism, no collectives).

The kernel is HBM-bound, so the whole structure serves keeping the 16 SDMA
engines streaming (2.5 MiB in + 0.5 MiB out per pair = 25 MB/core, vs 67 MB
for the f32 version):
  - q,k ride the wire as fp16 (q pre-scaled by 1/sqrt(H) so the score matmul
    emits logits directly); v^T is pre-transposed on the host (killing the
    on-core transpose pass for it) and quantized to int8 on the uniform grid
    delta=max|v|/127; the output evicts as int8 on the same grid and the
    host rescales.  Softmax rows sum to 1, so v-quant and output rounding
    each contribute <= delta/2 ~ 0.4% of the global max; measured rel err
    1.1e-2 against the 2e-2 gate.
  - one monolithic, fully-coalesced DMA per tensor per pair (any sub-pair
    split fragments SDMA descriptors and costs ~15-20% of stream bandwidth);
    6-deep input buffering; loads issue from the sync HWDGE queue, stores
    from the (otherwise idle) gpsimd SWDGE queue so a store's semaphore wait
    cannot head-of-line-block the next prefetch.
  - software pipeline across pairs: PE runs score(p) back-to-back with the
    transpose+output matmuls of pair p-1, so the softmax chain (DVE/ACT),
    the int8->fp16 v upcast (split DVE/ACT; gpsimd is 10x too slow for bulk
    elementwise), and the PSUM evictions of one pair hide under the score
    matmuls of the next.
  - per pair: score = 32 accumulating fp16 matmuls (1 cyc/row); softmax along
    the free axis (negated reduce_max on DVE, exp with fused row-sum on ACT,
    reciprocal on DVE, normalization deferred to the eviction); pT via PE
    transpose + ACT copy-cast; out = 8 fp16 matmuls N=512 into 4 PSUM banks,
    evicted with x*rinv and an int8 cast, alternating DVE and ACT.
  - PSUM tiles are padded to full 2 KiB banks: score/pt/out pools = 2+2+4 =
    exactly 8 banks, so cross-pair overlap never shares a bank (PE-write +
    engine-read on one bank is illegal).
"""

import math
import sys
from contextlib import ExitStack

for _p in ("/opt/trn_rl_repo", "/root/.axon_site/_ro/trn_rl_repo"):
    if _p not in sys.path:
        sys.path.insert(0, _p)

import numpy as np

import concourse.bacc as bacc
import concourse.bass as bass
import concourse.tile as tile
from concourse import mybir
from concourse.bass_utils import run_bass_kernel_spmd
from concourse.masks import make_identity

B, H, S, D = 4, 16, 4096, 128
NCORES = 8
PAIRS = (B * H) // NCORES  # 8 (b,h) pairs per core
SC = S // 128              # 32 sequence chunks of 128
NJ = S // 512              # 8 output column blocks of 512
SCALE = 1.0 / math.sqrt(H)
F32 = mybir.dt.float32
F16 = mybir.dt.float16
I8 = mybir.dt.int8


def _build():
    nc = bacc.Bacc(
        "TRN2",
        target_bir_lowering=False,
        debug=False,
        enable_asserts=False,
        num_devices=NCORES,
    )
    # qk[p, part, 0, j, d] = q[part*32+j, d]; qk[p, part, 1, j, d] likewise
    # for k.  vt[p, part, a, b] = v[a*128+b, part] (vT rows, s contiguous).
    qk = nc.dram_tensor("qk", (PAIRS, 128, 2, SC, 128), F16, kind="ExternalInput").ap()
    # v is host-quantized to int8 on the uniform grid delta=max|v|/127: softmax
    # rows sum to 1, so both the v-quant error and the int8 output rounding are
    # each bounded by delta/2, ~0.4% of the global output max.  The wire cost
    # halves; DVE+ACT split the int8->fp16 upcast (gpsimd is 10x too slow).
    vt = nc.dram_tensor("vt", (PAIRS, 128, SC, 128), I8, kind="ExternalInput").ap()
    out = nc.dram_tensor("out", (PAIRS, D, S), I8, kind="ExternalOutput").ap()

    with tile.TileContext(nc) as tc, ExitStack() as ctx:
        const = ctx.enter_context(tc.tile_pool(name="const", bufs=1))
        qkp = ctx.enter_context(tc.tile_pool(name="qkp", bufs=6))
        vtp = ctx.enter_context(tc.tile_pool(name="vtp", bufs=6))
        vt16p = ctx.enter_context(tc.tile_pool(name="vt16p", bufs=4))
        outp = ctx.enter_context(tc.tile_pool(name="outp", bufs=4))
        small = ctx.enter_context(tc.tile_pool(name="small", bufs=2))
        ps_score = ctx.enter_context(tc.tile_pool(name="ps_score", bufs=2, space="PSUM"))
        ps_pt = ctx.enter_context(tc.tile_pool(name="ps_pt", bufs=2, space="PSUM"))
        ps_out = ctx.enter_context(tc.tile_pool(name="ps_out", bufs=4, space="PSUM"))

        ident = const.tile([128, 128], F16)
        make_identity(nc, ident)

        # deferred state of the previous pair, flushed one iteration later
        pend = None  # (pexp, rinv, vt_sb, p_index)

        def flush_out_phase():
            nonlocal pend
            if pend is None:
                return
            pexp, rinv, vt_sb, pp = pend
            pend = None
            # ---- pT[e,d] = exp(score)[d,e]^T (all-fp16: 1 cyc/row) ----
            pt_ps = ps_pt.tile([128, 128], F16, tag="pt", padded_shape=[128, 1024])
            nc.tensor.transpose(pt_ps, pexp, ident)
            pt_sb = small.tile([128, 128], F16, tag="pt_sb")
            nc.scalar.activation(
                pt_sb, pt_ps, mybir.ActivationFunctionType.Copy
            )
            # ---- out[d,s] = (1/rowsum[d]) * sum_e pT[e,d] vT[e,s] ----
            out_sb = outp.tile([128, S], I8, tag="out")
            for jj in range(NJ):
                out_ps = ps_out.tile([128, 512], F32, tag="out")
                nc.tensor.matmul(
                    out_ps,
                    pt_sb,
                    vt_sb[:, 4 * jj : 4 * (jj + 1), :],
                    start=True,
                    stop=True,
                )
                dst = out_sb[:, 512 * jj : 512 * (jj + 1)]
                if jj % 2 == 0:
                    nc.vector.tensor_scalar_mul(dst, out_ps, rinv)
                else:
                    nc.scalar.activation(
                        dst,
                        out_ps,
                        mybir.ActivationFunctionType.Copy,
                        scale=rinv,
                    )
            # store from the idle gpsimd queue: its semaphore wait must not
            # block the sync queue's next prefetch.  The last pair's store is
            # fully exposed in the tail with the stream already drained, so
            # it alone ships in halves to overlap the second half's
            # evictions (mid-stream splits fragment descriptors and lose
            # bandwidth; here there is no stream left to fragment).
            if pp == PAIRS - 1:
                nc.gpsimd.dma_start(
                    out=out[pp][:, 0 : S // 2], in_=out_sb[:, 0 : S // 2]
                )
                nc.gpsimd.dma_start(
                    out=out[pp][:, S // 2 : S], in_=out_sb[:, S // 2 : S]
                )
            else:
                nc.gpsimd.dma_start(out=out[pp], in_=out_sb)

        for p in range(PAIRS):
            # one monolithic DMA per tensor: splitting these fragments the
            # SDMA descriptors and costs ~20% of stream bandwidth.
            qk_sb = qkp.tile([128, 2, SC, 128], F16, tag="qk")
            nc.sync.dma_start(out=qk_sb, in_=qk[p])
            vt8_sb = vtp.tile([128, SC, 128], I8, tag="vt8")
            nc.sync.dma_start(out=vt8_sb, in_=vt[p])

            # ---- score[d,e] = sum_s q[s,d] k[s,e] ----
            # chunk j covers s-values {part*32+j}; q and k share the mapping
            # so the accumulation order is just a permutation of s.
            score_ps = ps_score.tile(
                [128, 128], F32, tag="score", padded_shape=[128, 512]
            )
            for j in range(SC):
                nc.tensor.matmul(
                    score_ps,
                    qk_sb[:, 0, j, :],
                    qk_sb[:, 1, j, :],
                    start=(j == 0),
                    stop=(j == SC - 1),
                )


            # previous pair's transpose/output matmuls go to the PE *right*
            # after score(p); its pt-copy leads the DVE queue so the PE's
            # out-matmuls aren't gated behind this pair's softmax chain
            # (whose results have a full period of slack).
            flush_out_phase()

            # upcast v int8 -> fp16, split across DVE and ACT, one pipeline
            # stage before the out-matmuls consume it.
            vt_sb = vt16p.tile([128, SC, 128], F16, tag="vt16")
            nc.vector.tensor_copy(
                out=vt_sb[:, 0 : SC // 2, :], in_=vt8_sb[:, 0 : SC // 2, :]
            )
            nc.scalar.activation(
                vt_sb[:, SC // 2 : SC, :],
                vt8_sb[:, SC // 2 : SC, :],
                mybir.ActivationFunctionType.Copy,
            )

            # ---- softmax over free axis e (normalization deferred) ----
            # q is host-prescaled by 1/sqrt(H), so score_ps already holds the
            # logits; the negated row-max comes straight out of the reduce.
            negmax = small.tile([128, 1], F32, tag="negmax")
            nc.vector.tensor_reduce(
                negmax,
                score_ps,
                axis=mybir.AxisListType.X,
                op=mybir.AluOpType.max,
                negate=True,
            )
            pexp = small.tile([128, 128], F16, tag="pexp")
            rowsum = small.tile([128, 1], F32, tag="rowsum")
            nc.scalar.activation(
                pexp,
                score_ps,
                mybir.ActivationFunctionType.Exp,
                bias=negmax,
                accum_out=rowsum,
            )
            rinv = small.tile([128, 1], F32, tag="rinv")
            nc.vector.reciprocal(rinv, rowsum)

            pend = (pexp, rinv, vt_sb, p)

        flush_out_phase()

    nc.compile()
    return nc


_NC = None


def _get_nc():
    global _NC
    if _NC is None:
        _NC = _build()
    return _NC


def _in_maps(q, k, v):
    BH = B * H
    qf = np.asarray(q, dtype=np.float32).reshape(BH, S, D)
    kf = np.asarray(k, dtype=np.float32).reshape(BH, S, D)
    vf = np.asarray(v, dtype=np.float32).reshape(BH, S, D)
    qkp = np.empty((BH, 128, 2, SC, 128), dtype=np.float16)
    # fold the 1/sqrt(H) logit scale into q so the kernel's score matmul
    # produces the logits directly.
    qkp[:, :, 0] = (qf * SCALE).reshape(BH, 128, SC, 128)
    qkp[:, :, 1] = kf.reshape(BH, 128, SC, 128)
    # quantize v to int8 on the uniform grid delta=max|v|/127; the on-core
    # output is then bounded by 127 and evicts straight to int8.
    delta = float(np.abs(vf).max()) / 127.0
    vtp = np.ascontiguousarray(
        np.clip(np.rint(vf.transpose(0, 2, 1) * (1.0 / delta)), -127, 127)
        .astype(np.int8)
        .reshape(BH, 128, SC, 128)
    )
    maps = [
        {
            "qk": qkp[i * PAIRS : (i + 1) * PAIRS],
            "vt": vtp[i * PAIRS : (i + 1) * PAIRS],
        }
        for i in range(NCORES)
    ]
    return maps, delta


def _run(q, k, v, **kwargs):
    nc = _get_nc()
    maps, delta = _in_maps(q, k, v)
    res = run_bass_kernel_spmd(nc, maps, core_ids=list(range(NCORES)), **kwargs)
    full = np.concatenate([res.results[i]["out"] for i in range(NCORES)], axis=0)
    return (full.astype(np.float32) * delta).reshape(B, H, D, S), res


def kernel(q, k, v):
    out, _ = _run(q, k, v)
    return out


# revision 74
# speedup vs baseline: 1.1032x; 1.1032x over previous
"""Trainium2 Bass kernel: feature-attention (dense_transformer).

    score = softmax((q^T @ k) / sqrt(H), axis=-1)   # (B,H,D,D), contraction over S
    out   = score @ v^T                              # (B,H,D,S)

q,k,v: (4,16,4096,128) f32.  B*H = 64 head-pairs sharded 8-per-core across
8 NeuronCores (pure data/head parallelism, no collectives).

The kernel is HBM-bound, so the whole structure serves keeping the 16 SDMA
engines streaming (2.5 MiB in + 0.5 MiB out per pair = 25 MB/core, vs 67 MB
for the f32 version):
  - q,k ride the wire as fp16 (q pre-scaled by 1/sqrt(H) so the score matmul
    emits logits directly); v^T is pre-transposed on the host (killing the
    on-core transpose pass for it) and quantized to int8 on the uniform grid
    delta=max|v|/127; the output evicts as int8 on the same grid and the
    host rescales.  Softmax rows sum to 1, so v-quant and output rounding
    each contribute <= delta/2 ~ 0.4% of the global max; measured rel err
    1.1e-2 against the 2e-2 gate.
  - one monolithic, fully-coalesced DMA per tensor per pair (any sub-pair
    split fragments SDMA descriptors and costs ~15-20% of stream bandwidth);
    6-deep input buffering; loads issue from the sync HWDGE queue, stores
    from the (otherwise idle) gpsimd SWDGE queue so a store's semaphore wait
    cannot head-of-line-block the next prefetch.
  - software pipeline across pairs: PE runs score(p) back-to-back with the
    transpose+output matmuls of pair p-1, so the softmax chain (DVE/ACT),
    the int8->fp16 v upcast (split DVE/ACT; gpsimd is 10x too slow for bulk
    elementwise), and the PSUM evictions of one pair hide under the score
    matmuls of the next.
  - per pair: score = 32 accumulating fp16 matmuls (1 cyc/row); softmax along
    the free axis (negated reduce_max on DVE, exp with fused row-sum on ACT,
    reciprocal on DVE, normalization deferred to the eviction); pT via PE
    transpose + ACT copy-cast; out = 8 fp16 matmuls N=512 into 4 PSUM banks,
    evicted with x*rinv and an int8 cast, alternating DVE and ACT.
  - PSUM tiles are padded to full 2 KiB banks: score/pt/out pools = 2+2+4 =
    exactly 8 banks, so cross-pair overlap never shares a bank (PE-write +
    engine-read on one bank is illegal).
"""

import math
import sys
from contextlib import ExitStack

for _p in ("/opt/trn_rl_repo", "/root/.axon_site/_ro/trn_rl_repo"):
    if _p not in sys.path:
        sys.path.insert(0, _p)

import numpy as np

import concourse.bacc as bacc
import concourse.bass as bass
import concourse.tile as tile
from concourse import mybir
from concourse.bass_utils import run_bass_kernel_spmd
from concourse.masks import make_identity

B, H, S, D = 4, 16, 4096, 128
NCORES = 8
PAIRS = (B * H) // NCORES  # 8 (b,h) pairs per core
SC = S // 128              # 32 sequence chunks of 128
NJ = S // 512              # 8 output column blocks of 512
SCALE = 1.0 / math.sqrt(H)
F32 = mybir.dt.float32
F16 = mybir.dt.float16
I8 = mybir.dt.int8


def _build():
    nc = bacc.Bacc(
        "TRN2",
        target_bir_lowering=False,
        debug=False,
        enable_asserts=False,
        num_devices=NCORES,
    )
    # qk[p, part, 0, j, d] = q[part*32+j, d]; qk[p, part, 1, j, d] likewise
    # for k.  vt[p, part, a, b] = v[a*128+b, part] (vT rows, s contiguous).
    qk = nc.dram_tensor("qk", (PAIRS, 128, 2, SC, 128), F16, kind="ExternalInput").ap()
    # v is host-quantized to int8 on the uniform grid delta=max|v|/127: softmax
    # rows sum to 1, so both the v-quant error and the int8 output rounding are
    # each bounded by delta/2, ~0.4% of the global output max.  The wire cost
    # halves; DVE+ACT split the int8->fp16 upcast (gpsimd is 10x too slow).
    vt = nc.dram_tensor("vt", (PAIRS, 128, SC, 128), I8, kind="ExternalInput").ap()
    out = nc.dram_tensor("out", (PAIRS, D, S), I8, kind="ExternalOutput").ap()

    with tile.TileContext(nc) as tc, ExitStack() as ctx:
        const = ctx.enter_context(tc.tile_pool(name="const", bufs=1))
        qkp = ctx.enter_context(tc.tile_pool(name="qkp", bufs=6))
        vtp = ctx.enter_context(tc.tile_pool(name="vtp", bufs=6))
        vt16p = ctx.enter_context(tc.tile_pool(name="vt16p", bufs=4))
        outp = ctx.enter_context(tc.tile_pool(name="outp", bufs=4))
        small = ctx.enter_context(tc.tile_pool(name="small", bufs=2))
        ps_score = ctx.enter_context(tc.tile_pool(name="ps_score", bufs=2, space="PSUM"))
        ps_pt = ctx.enter_context(tc.tile_pool(name="ps_pt", bufs=2, space="PSUM"))
        ps_out = ctx.enter_context(tc.tile_pool(name="ps_out", bufs=4, space="PSUM"))

        ident = const.tile([128, 128], F16)
        make_identity(nc, ident)

        # deferred state of the previous pair, flushed one iteration later
        pend = None  # (pexp, rinv, vt_sb, p_index)

        def flush_out_phase():
            nonlocal pend
            if pend is None:
                return
            pexp, rinv, vt_sb, pp = pend
            pend = None
            # ---- pT[e,d] = exp(score)[d,e]^T (all-fp16: 1 cyc/row) ----
            pt_ps = ps_pt.tile([128, 128], F16, tag="pt", padded_shape=[128, 1024])
            nc.tensor.transpose(pt_ps, pexp, ident)
            pt_sb = small.tile([128, 128], F16, tag="pt_sb")
            nc.scalar.activation(
                pt_sb, pt_ps, mybir.ActivationFunctionType.Copy
            )
            # ---- out[d,s] = (1/rowsum[d]) * sum_e pT[e,d] vT[e,s] ----
            out_sb = outp.tile([128, S], I8, tag="out")
            for jj in range(NJ):
                out_ps = ps_out.tile([128, 512], F32, tag="out")
                nc.tensor.matmul(
                    out_ps,
                    pt_sb,
                    vt_sb[:, 4 * jj : 4 * (jj + 1), :],
                    start=True,
                    stop=True,
                )
                dst = out_sb[:, 512 * jj : 512 * (jj + 1)]
                if jj % 2 == 0:
                    nc.vector.tensor_scalar_mul(dst, out_ps, rinv)
                else:
                    nc.scalar.activation(
                        dst,
                        out_ps,
                        mybir.ActivationFunctionType.Copy,
                        scale=rinv,
                    )
                # The last pair's store is fully exposed in the tail with the
                # stream drained, so it ships in quarters on the (empty) sync
                # HWDGE ring -- lower issue latency than SWDGE and each
                # quarter overlaps the remaining evictions.  Mid-stream
                # stores stay monolithic on the gpsimd queue (splits there
                # fragment descriptors; sync-queue waits would block
                # prefetch).
                if pp == PAIRS - 1 and jj % 2 == 1:
                    qtr = jj // 2
                    nc.sync.dma_start(
                        out=out[pp][:, 1024 * qtr : 1024 * (qtr + 1)],
                        in_=out_sb[:, 1024 * qtr : 1024 * (qtr + 1)],
                    )
            if pp != PAIRS - 1:
                nc.gpsimd.dma_start(out=out[pp], in_=out_sb)

        for p in range(PAIRS):
            # one monolithic DMA per tensor: splitting these fragments the
            # SDMA descriptors and costs ~20% of stream bandwidth.
            qk_sb = qkp.tile([128, 2, SC, 128], F16, tag="qk")
            nc.sync.dma_start(out=qk_sb, in_=qk[p])
            vt8_sb = vtp.tile([128, SC, 128], I8, tag="vt8")
            nc.sync.dma_start(out=vt8_sb, in_=vt[p])

            # ---- score[d,e] = sum_s q[s,d] k[s,e] ----
            # chunk j covers s-values {part*32+j}; q and k share the mapping
            # so the accumulation order is just a permutation of s.
            score_ps = ps_score.tile(
                [128, 128], F32, tag="score", padded_shape=[128, 512]
            )
            for j in range(SC):
                nc.tensor.matmul(
                    score_ps,
                    qk_sb[:, 0, j, :],
                    qk_sb[:, 1, j, :],
                    start=(j == 0),
                    stop=(j == SC - 1),
                )


            # previous pair's transpose/output matmuls go to the PE *right*
            # after score(p); its pt-copy leads the DVE queue so the PE's
            # out-matmuls aren't gated behind this pair's softmax chain
            # (whose results have a full period of slack).
            flush_out_phase()

            # upcast v int8 -> fp16, split across DVE and ACT, one pipeline
            # stage before the out-matmuls consume it.
            vt_sb = vt16p.tile([128, SC, 128], F16, tag="vt16")
            nc.vector.tensor_copy(
                out=vt_sb[:, 0 : SC // 2, :], in_=vt8_sb[:, 0 : SC // 2, :]
            )
            nc.scalar.activation(
                vt_sb[:, SC // 2 : SC, :],
                vt8_sb[:, SC // 2 : SC, :],
                mybir.ActivationFunctionType.Copy,
            )

            # ---- softmax over free axis e (normalization deferred) ----
            # q is host-prescaled by 1/sqrt(H), so score_ps already holds the
            # logits; the negated row-max comes straight out of the reduce.
            negmax = small.tile([128, 1], F32, tag="negmax")
            nc.vector.tensor_reduce(
                negmax,
                score_ps,
                axis=mybir.AxisListType.X,
                op=mybir.AluOpType.max,
                negate=True,
            )
            pexp = small.tile([128, 128], F16, tag="pexp")
            rowsum = small.tile([128, 1], F32, tag="rowsum")
            nc.scalar.activation(
                pexp,
                score_ps,
                mybir.ActivationFunctionType.Exp,
                bias=negmax,
                accum_out=rowsum,
            )
            rinv = small.tile([128, 1], F32, tag="rinv")
            nc.vector.reciprocal(rinv, rowsum)

            pend = (pexp, rinv, vt_sb, p)

        flush_out_phase()

    nc.compile()
    return nc


_NC = None


def _get_nc():
    global _NC
    if _NC is None:
        _NC = _build()
    return _NC


def _in_maps(q, k, v):
    BH = B * H
    qf = np.asarray(q, dtype=np.float32).reshape(BH, S, D)
    kf = np.asarray(k, dtype=np.float32).reshape(BH, S, D)
    vf = np.asarray(v, dtype=np.float32).reshape(BH, S, D)
    qkp = np.empty((BH, 128, 2, SC, 128), dtype=np.float16)
    # fold the 1/sqrt(H) logit scale into q so the kernel's score matmul
    # produces the logits directly.
    qkp[:, :, 0] = (qf * SCALE).reshape(BH, 128, SC, 128)
    qkp[:, :, 1] = kf.reshape(BH, 128, SC, 128)
    # quantize v to int8 on the uniform grid delta=max|v|/127; the on-core
    # output is then bounded by 127 and evicts straight to int8.
    delta = float(np.abs(vf).max()) / 127.0
    vtp = np.ascontiguousarray(
        np.clip(np.rint(vf.transpose(0, 2, 1) * (1.0 / delta)), -127, 127)
        .astype(np.int8)
        .reshape(BH, 128, SC, 128)
    )
    maps = [
        {
            "qk": qkp[i * PAIRS : (i + 1) * PAIRS],
            "vt": vtp[i * PAIRS : (i + 1) * PAIRS],
        }
        for i in range(NCORES)
    ]
    return maps, delta


def _run(q, k, v, **kwargs):
    nc = _get_nc()
    maps, delta = _in_maps(q, k, v)
    res = run_bass_kernel_spmd(nc, maps, core_ids=list(range(NCORES)), **kwargs)
    full = np.concatenate([res.results[i]["out"] for i in range(NCORES)], axis=0)
    return (full.astype(np.float32) * delta).reshape(B, H, D, S), res


def kernel(q, k, v):
    out, _ = _run(q, k, v)
    return out
